# revision 6
# baseline (speedup 1.0000x reference)
"""DeepseekV4 Indexer kernel for 8x TRN2 NeuronCores (Bass/Tile).

Sequence-parallel sharding: core c owns tokens [c*512, (c+1)*512).
Each core: computes the full block-compressor (redundantly), its token
chunk's q-projection + rope + per-head ReLU scores + head-weighted sum,
causal mask, and a top-512 sort (merge network on float keys with
embedded indices). Host assembles the full outputs.

Precision modes (env BASS_KERNEL_PREC):
  split (default): all matmuls as 3-pass bf16 hi/lo split (fp32-class).
  f32r:            1-pass float32r (fast, ~12-bit mantissa).
"""
import os
import numpy as np
import ml_dtypes
from contextlib import ExitStack

import concourse.bass as bass
import concourse.mybir as mybir
import concourse.tile as tile
from concourse import bacc
import concourse.bass_utils as bass_utils

F32 = mybir.dt.float32
F32R = mybir.dt.float32r
BF16 = mybir.dt.bfloat16
U8 = mybir.dt.uint8
U32 = mybir.dt.uint32
I32 = mybir.dt.int32
ALU = mybir.AluOpType
ACTF = mybir.ActivationFunctionType

B, S, H = 1, 4096, 4096
NH, DH, ROPE_D = 64, 128, 64
RR, TOPK = 4, 512
NC = S // RR            # 1024 blocks
NCORES = 8
TC = S // NCORES        # 512 tokens per core
NTT = TC // 128         # 4 token tiles per core
KT = H // 128           # 32 contraction tiles
NSL = (NH * DH) // 512  # 16 dh slices of 512 (4 heads each)
THETA_Q, THETA_K = 10000.0, 160000.0
EPS = 1e-6

PREC = os.environ.get("BASS_KERNEL_PREC", "split")

_cache = {}


def _bf16_split(x):
    hi = x.astype(ml_dtypes.bfloat16)
    lo = (x - hi.astype(np.float32)).astype(ml_dtypes.bfloat16)
    return hi, lo


def _rne12(x):
    x = np.ascontiguousarray(x, dtype=np.float32)
    u = x.view(np.uint32).astype(np.int64)
    low = u & 4095
    base = u & ~4095
    inc = (low > 2048) | ((low == 2048) & (((u >> 12) & 1) == 1))
    return ((base + np.where(inc, 4096, 0)) & 0xFFFFFFFF).astype(np.uint32).view(np.float32)


def _cos_sin(pos, theta):
    # 32 distinct frequencies for the 64 rope dims (each repeated twice)
    inv = 1.0 / (theta ** (np.arange(0, ROPE_D, 2, dtype=np.float64) / ROPE_D))
    ang = pos.astype(np.float64)[:, None] * inv
    return np.cos(ang).astype(np.float32), np.sin(ang).astype(np.float32)


def _build_program(prec):
    nc = bacc.Bacc("TRN2", target_bir_lowering=False, debug=False,
                   enable_asserts=False, num_devices=NCORES)
    split = prec == "split"
    wdt = BF16 if split else F32R

    def din(name, shape, dtype):
        return nc.dram_tensor(name, shape, dtype, kind="ExternalInput").ap()

    # weights / shared
    if split:
        wq_hi = din("wq_hi", [128, NSL, KT, 512], BF16)
        wq_lo = din("wq_lo", [128, NSL, KT, 512], BF16)
        hs_hi = din("hs_hi", [128, KT, 8, 512], BF16)
        hs_lo = din("hs_lo", [128, KT, 8, 512], BF16)
        qr_hi = din("qr_hi", [128, KT, 512], BF16)
        qr_lo = din("qr_lo", [128, KT, 512], BF16)
        wkv_hi = din("wkv_hi", [128, KT * 128], BF16)
        wkv_lo = din("wkv_lo", [128, KT * 128], BF16)
        wg_hi = din("wg_hi", [128, KT * 128], BF16)
        wg_lo = din("wg_lo", [128, KT * 128], BF16)
        ww_hi = din("ww_hi", [128, KT * 64], BF16)
        ww_lo = din("ww_lo", [128, KT * 64], BF16)
    else:
        wq_hi = din("wq_hi", [128, NSL, KT, 512], F32R)
        hs_hi = din("hs_hi", [128, KT, 8, 512], F32R)
        qr_hi = din("qr_hi", [128, KT, 512], F32R)
        wkv_hi = din("wkv_hi", [128, KT * 128], F32R)
        wg_hi = din("wg_hi", [128, KT * 128], F32R)
        ww_hi = din("ww_hi", [128, KT * 64], F32R)

    apet_d = din("apet", [128, 4], F32)
    kvnw_d = din("kvnw", [128, 128], F32)
    ident_d = din("ident", [128, 128], F32)
    cosk_d = din("cosk", [128, 8, 32], F32)
    sink_d = din("sink", [128, 8, 32], F32)
    cosq_d = din("cosq", [128, NTT, 32], F32)
    sinq_d = din("sinq", [128, NTT, 32], F32)
    p4_d = din("p4", [128, NC], F32)
    tpos_d = din("tpos", [128, NC], U32)
    tneg_d = din("tneg", [128, NC], U32)
    tglob_d = din("tglob", [128, NTT], F32)

    scores_o = nc.dram_tensor("scores_o", [TC, NC], F32, kind="ExternalOutput").ap()
    topk_o = nc.dram_tensor("topk_o", [TC, TOPK], I32, kind="ExternalOutput").ap()

    with tile.TileContext(nc) as tc, ExitStack() as ctx:
        persist = ctx.enter_context(tc.tile_pool(name="persist", bufs=1))

        # ---- persistent tiles ----
        def pload(dram, shape, dtype, tag):
            t = persist.tile(shape, dtype, tag=tag, name=tag)
            nc.sync.dma_start(t[:], dram[:])
            return t

        apet = pload(apet_d, [128, 4], F32, "apet")
        kvnw = pload(kvnw_d, [128, 128], F32, "kvnw")
        ident = pload(ident_d, [128, 128], F32, "ident")
        cosk = pload(cosk_d, [128, 8, 32], F32, "cosk")
        sink = pload(sink_d, [128, 8, 32], F32, "sink")
        cosq = pload(cosq_d, [128, NTT, 32], F32, "cosq")
        sinq = pload(sinq_d, [128, NTT, 32], F32, "sinq")
        p4 = pload(p4_d, [128, NC], F32, "p4")
        tpos = pload(tpos_d, [128, NC], U32, "tpos")
        tneg = pload(tneg_d, [128, NC], U32, "tneg")
        tglob = pload(tglob_d, [128, NTT], F32, "tglob")

        qrh = pload(qr_hi, [128, KT, 512], wdt, "qrh")
        qrl = None
        if split:
            qrl = pload(qr_lo, [128, KT, 512], BF16, "qrl")

        neg = persist.tile([128, NC], F32, tag="neg")
        nc.vector.memset(neg[:], -1e9)
        epsb = persist.tile([128, 1], F32, tag="epsb")
        nc.vector.memset(epsb[:], EPS)

        kth = persist.tile([128, NC], wdt, tag="kth")
        ktl = None
        if split:
            ktl = persist.tile([128, NC], BF16, tag="ktl", name="ktl")

        acc = [persist.tile([128, NC], F32, tag=f"acc{t}", name=f"acc{t}") for t in range(NTT)]
        for t in range(NTT):
            nc.vector.memset(acc[t][:], 0.0)

        wabs = [persist.tile([128, NH], F32, tag=f"wabs{t}", name=f"wabs{t}") for t in range(NTT)]
        wsgn = [persist.tile([128, NH], F32, tag=f"wsgn{t}", name=f"wsgn{t}") for t in range(NTT)]

        WSCALE = float(DH ** -0.5) * float(NH ** -0.5)

        # ---- w projection: w = q_res @ w_weights ----
        with tc.tile_pool(name="wproj_ps", bufs=2, space="PSUM") as wps, \
             tc.tile_pool(name="wproj_sb", bufs=1) as wsb:
            wwh = wsb.tile([128, KT * 64], wdt, tag="wwh")
            nc.sync.dma_start(wwh[:], ww_hi[:])
            if split:
                wwl = wsb.tile([128, KT * 64], BF16, tag="wwl")
                nc.sync.dma_start(wwl[:], ww_lo[:])
            for t in range(NTT):
                psw = wps.tile([128, NH], F32, tag="psw")
                terms = [(qrh, wwh), (qrh, wwl), (qrl, wwh)] if split else [(qrh, wwh)]
                for ti, (qa, wb) in enumerate(terms):
                    for k in range(KT):
                        nc.tensor.matmul(
                            psw[:], qa[:, k, t * 128:(t + 1) * 128],
                            wb[:, k * 64:(k + 1) * 64],
                            start=(ti == 0 and k == 0),
                            stop=(ti == len(terms) - 1 and k == KT - 1))
                wsc = wsb.tile([128, NH], F32, tag="wsc")
                nc.vector.tensor_scalar(wsc[:], psw[:], WSCALE, scalar2=None, op0=ALU.mult)
                m = wsb.tile([128, NH], F32, tag="wsgnm")
                nc.vector.tensor_scalar(m[:], wsc[:], 0.0, scalar2=None, op0=ALU.is_ge)
                nc.vector.tensor_scalar(wsgn[t][:], m[:], 2.0, scalar2=1.0, op0=ALU.mult, op1=ALU.subtract)
                nc.vector.tensor_tensor(wabs[t][:], wsc[:], wsgn[t][:], ALU.mult)

        # ---- compressor: pooled keys (all 1024 blocks) ----
        with tc.tile_pool(name="comp_ps", bufs=2, space="PSUM") as cps, \
             tc.tile_pool(name="comp_tr", bufs=2, space="PSUM") as ctr, \
             tc.tile_pool(name="comp_sb", bufs=1) as csb, \
             tc.tile_pool(name="comp_stream", bufs=4) as cstr, \
             tc.tile_pool(name="comp_scr", bufs=2) as cscr:
            wkvh = csb.tile([128, KT * 128], wdt, tag="wkvh")
            nc.sync.dma_start(wkvh[:], wkv_hi[:])
            wgh = csb.tile([128, KT * 128], wdt, tag="wgh")
            nc.sync.dma_start(wgh[:], wg_hi[:])
            if split:
                wkvl = csb.tile([128, KT * 128], BF16, tag="wkvl")
                nc.sync.dma_start(wkvl[:], wkv_lo[:])
                wgl = csb.tile([128, KT * 128], BF16, tag="wgl")
                nc.sync.dma_start(wgl[:], wg_lo[:])

            for tb in range(8):
                psk = cps.tile([128, 512], F32, tag="psk")
                psg = cps.tile([128, 512], F32, tag="psg")
                for k in range(KT):
                    hst_h = cstr.tile([128, 512], wdt, tag="hsth")
                    nc.sync.dma_start(hst_h[:], hs_hi[:, k, tb, :])
                    if split:
                        hst_l = cstr.tile([128, 512], BF16, tag="hstl")
                        nc.sync.dma_start(hst_l[:], hs_lo[:, k, tb, :])
                    wk = wkvh[:, k * 128:(k + 1) * 128]
                    wg_ = wgh[:, k * 128:(k + 1) * 128]
                    last = (k == KT - 1)
                    nc.tensor.matmul(psk[:], wk, hst_h[:], start=(k == 0),
                                     stop=(last and not split))
                    nc.tensor.matmul(psg[:], wg_, hst_h[:], start=(k == 0),
                                     stop=(last and not split))
                    if split:
                        wkl = wkvl[:, k * 128:(k + 1) * 128]
                        wgl_ = wgl[:, k * 128:(k + 1) * 128]
                        nc.tensor.matmul(psk[:], wk, hst_l[:], start=False, stop=False)
                        nc.tensor.matmul(psk[:], wkl, hst_h[:], start=False, stop=last)
                        nc.tensor.matmul(psg[:], wg_, hst_l[:], start=False, stop=False)
                        nc.tensor.matmul(psg[:], wgl_, hst_h[:], start=False, stop=last)

                # pooling: softmax over r (groups of 4 along free axis)
                gb = cscr.tile([128, 512], F32, tag="gb")
                gv = psg[:].rearrange("p (b r) -> p b r", r=4)
                apev = apet[:].rearrange("p (o r) -> p o r", o=1).to_broadcast([128, 128, 4])
                nc.vector.tensor_tensor(gb[:].rearrange("p (b r) -> p b r", r=4), gv, apev, ALU.add)
                et = cscr.tile([128, 512], F32, tag="et")
                nc.scalar.activation(et[:], gb[:], ACTF.Exp)
                esum = cscr.tile([128, 128], F32, tag="esum")
                nc.vector.tensor_reduce(esum[:], et[:].rearrange("p (b r) -> p b r", r=4),
                                        op=ALU.add, axis=mybir.AxisListType.X)
                kve = cscr.tile([128, 512], F32, tag="kve")
                nc.vector.tensor_tensor(kve[:], psk[:], et[:], ALU.mult)
                kvs = cscr.tile([128, 128], F32, tag="kvs")
                nc.vector.tensor_reduce(kvs[:], kve[:].rearrange("p (b r) -> p b r", r=4),
                                        op=ALU.add, axis=mybir.AxisListType.X)
                erec = cscr.tile([128, 128], F32, tag="erec")
                nc.vector.reciprocal(erec[:], esum[:])
                pooled = cscr.tile([128, 128], F32, tag="pooled")
                nc.vector.tensor_tensor(pooled[:], kvs[:], erec[:], ALU.mult)

                # transpose to [blk, d]
                pst = ctr.tile([128, 128], F32, tag="pst")
                nc.tensor.transpose(pst[:], pooled[:], ident[:])
                pb = cscr.tile([128, 128], F32, tag="pb")
                nc.scalar.copy(pb[:], pst[:])

                # rmsnorm over d (free axis now)
                sqj = cscr.tile([128, 128], F32, tag="sqj")
                ssq = cscr.tile([128, 1], F32, tag="ssq")
                nc.vector.scalar_tensor_tensor(sqj[:], pb[:], 0.0, pb[:],
                                               op0=ALU.add, op1=ALU.mult, accum_out=ssq[:])
                rms = cscr.tile([128, 1], F32, tag="rms")
                nc.scalar.activation(rms[:], ssq[:], ACTF.Sqrt, bias=epsb[:], scale=float(1.0 / DH))
                rrec = cscr.tile([128, 1], F32, tag="rrec")
                nc.vector.reciprocal(rrec[:], rms[:])
                pn = cscr.tile([128, 128], F32, tag="pn")
                nc.vector.scalar_tensor_tensor(pn[:], pb[:], rrec[:], kvnw[:],
                                               op0=ALU.mult, op1=ALU.mult)

                # rope (pairs along free axis), theta_K, positions 4*blk
                kroped = cscr.tile([128, 128], F32, tag="kroped")
                nc.scalar.copy(kroped[:, 0:64], pn[:, 0:64])
                pv = pn[:, 64:128].rearrange("p (f two) -> p f two", two=2)
                kv_ = kroped[:, 64:128].rearrange("p (f two) -> p f two", two=2)
                ck = cosk[:, tb, :]
                sk = sink[:, tb, :]
                m1 = cscr.tile([128, 32], F32, tag="km1")
                m2 = cscr.tile([128, 32], F32, tag="km2")
                m3 = cscr.tile([128, 32], F32, tag="km3")
                m4 = cscr.tile([128, 32], F32, tag="km4")
                nc.vector.tensor_tensor(m1[:], pv[:, :, 0], ck, ALU.mult)
                nc.vector.tensor_tensor(m2[:], pv[:, :, 1], sk, ALU.mult)
                nc.vector.tensor_tensor(m3[:], pv[:, :, 1], ck, ALU.mult)
                nc.vector.tensor_tensor(m4[:], pv[:, :, 0], sk, ALU.mult)
                nc.vector.tensor_tensor(kv_[:, :, 0], m1[:], m2[:], ALU.subtract)
                nc.vector.tensor_tensor(kv_[:, :, 1], m3[:], m4[:], ALU.add)

                # transpose back to [d, blk] and split
                pst2 = ctr.tile([128, 128], F32, tag="pst2")
                nc.tensor.transpose(pst2[:], kroped[:], ident[:])
                nc.scalar.copy(kth[:, tb * 128:(tb + 1) * 128], pst2[:])
                if split:
                    nc.vector.tensor_tensor(ktl[:, tb * 128:(tb + 1) * 128], pst2[:],
                                            kth[:, tb * 128:(tb + 1) * 128], ALU.subtract)

        # ---- q projection + rope + transpose + scores + head accumulation ----
        with tc.tile_pool(name="q_ps", bufs=2, space="PSUM") as qps, \
             tc.tile_pool(name="q_tr", bufs=2, space="PSUM") as qtr, \
             tc.tile_pool(name="s_ps", bufs=3, space="PSUM") as sps, \
             tc.tile_pool(name="wq_stream", bufs=4) as wstr, \
             tc.tile_pool(name="q_sb", bufs=3) as qsb, \
             tc.tile_pool(name="q_scr", bufs=3) as qscr, \
             tc.tile_pool(name="relu_sb", bufs=3) as rsb:
            for n in range(NSL):
                # stream wq slice n (all 32 k tiles)
                wqt_h = []
                wqt_l = []
                for kc in range(0, KT, 8):
                    th = wstr.tile([128, 8, 512], wdt, tag="wqh", name="wqh")
                    nc.sync.dma_start(th[:], wq_hi[:, n, kc:kc + 8, :])
                    wqt_h.append(th)
                    if split:
                        tl = wstr.tile([128, 8, 512], BF16, tag="wql", name="wql")
                        nc.sync.dma_start(tl[:], wq_lo[:, n, kc:kc + 8, :])
                        wqt_l.append(tl)

                def wqh_(k):
                    return wqt_h[k // 8][:, k % 8, :]

                def wql_(k):
                    return wqt_l[k // 8][:, k % 8, :]

                for t in range(NTT):
                    psq = qps.tile([128, 512], F32, tag="psq")
                    for k in range(KT):
                        lh = qrh[:, k, t * 128:(t + 1) * 128]
                        nc.tensor.matmul(psq[:], lh, wqh_(k), start=(k == 0),
                                         stop=(k == KT - 1 and not split))
                        if split:
                            ll = qrl[:, k, t * 128:(t + 1) * 128]
                            nc.tensor.matmul(psq[:], lh, wql_(k), start=False, stop=False)
                            nc.tensor.matmul(psq[:], ll, wqh_(k), start=False,
                                             stop=(k == KT - 1))
                    # rope on the 4 heads of this slice (read PSUM, write SBUF)
                    q_sn = qsb.tile([128, 512], F32, tag="q_sn")
                    pvq = psq[:].rearrange("p (h d) -> p h d", h=4)
                    qvq = q_sn[:].rearrange("p (h d) -> p h d", h=4)
                    nc.scalar.copy(qvq[:, :, 0:64], pvq[:, :, 0:64])
                    x = pvq[:, :, 64:128].rearrange("p h (f two) -> p h f two", two=2)
                    y = qvq[:, :, 64:128].rearrange("p h (f two) -> p h f two", two=2)
                    cq = cosq[:, t, :].rearrange("p (o f) -> p o f", o=1).to_broadcast([128, 4, 32])
                    sq = sinq[:, t, :].rearrange("p (o f) -> p o f", o=1).to_broadcast([128, 4, 32])
                    r1 = qscr.tile([128, 128], F32, tag="r1")
                    r2 = qscr.tile([128, 128], F32, tag="r2")
                    r3 = qscr.tile([128, 128], F32, tag="r3")
                    r4 = qscr.tile([128, 128], F32, tag="r4")
                    r1v = r1[:].rearrange("p (h f) -> p h f", h=4)
                    r2v = r2[:].rearrange("p (h f) -> p h f", h=4)
                    r3v = r3[:].rearrange("p (h f) -> p h f", h=4)
                    r4v = r4[:].rearrange("p (h f) -> p h f", h=4)
                    nc.vector.tensor_tensor(r1v, x[:, :, :, 0], cq, ALU.mult)
                    nc.vector.tensor_tensor(r2v, x[:, :, :, 1], sq, ALU.mult)
                    nc.vector.tensor_tensor(r3v, x[:, :, :, 1], cq, ALU.mult)
                    nc.vector.tensor_tensor(r4v, x[:, :, :, 0], sq, ALU.mult)
                    nc.vector.tensor_tensor(y[:, :, :, 0], r1v, r2v, ALU.subtract)
                    nc.vector.tensor_tensor(y[:, :, :, 1], r3v, r4v, ALU.add)

                    for hh in range(4):
                        g = n * 4 + hh
                        pstq = qtr.tile([128, 128], F32, tag="pstq")
                        nc.tensor.transpose(pstq[:], q_sn[:, hh * 128:(hh + 1) * 128], ident[:])
                        qhi = qscr.tile([128, 128], wdt, tag="qhi")
                        nc.scalar.copy(qhi[:], pstq[:])
                        if split:
                            qlo = qscr.tile([128, 128], BF16, tag="qlo")
                            nc.vector.tensor_tensor(qlo[:], pstq[:], qhi[:], ALU.subtract)
                        for half in range(2):
                            pss = sps.tile([128, 512], F32, tag="pss")
                            ka = kth[:, half * 512:(half + 1) * 512]
                            nc.tensor.matmul(pss[:], qhi[:], ka, start=True, stop=not split)
                            if split:
                                kb = ktl[:, half * 512:(half + 1) * 512]
                                nc.tensor.matmul(pss[:], qhi[:], kb, start=False, stop=False)
                                nc.tensor.matmul(pss[:], qlo[:], ka, start=False, stop=True)
                            tmp = rsb.tile([128, 512], F32, tag="tmp")
                            nc.scalar.activation(tmp[:], pss[:], ACTF.Relu,
                                                 scale=wabs[t][:, g:g + 1])
                            ah = acc[t][:, half * 512:(half + 1) * 512]
                            nc.vector.scalar_tensor_tensor(ah, tmp[:], wsgn[t][:, g:g + 1], ah,
                                                           op0=ALU.mult, op1=ALU.add)

        # ---- finalize per token tile: mask, output scores, keys, sort, topk ----
        with tc.tile_pool(name="fin", bufs=2) as fin, \
             tc.tile_pool(name="keys", bufs=2) as kpool:
            for t in range(NTT):
                maskn = fin.tile([128, NC], U8, tag="maskn")
                nc.vector.tensor_scalar(maskn[:], p4[:], tglob[:, t:t + 1], scalar2=None,
                                        op0=ALU.is_gt)
                sco = fin.tile([128, NC], F32, tag="sco")
                nc.scalar.copy(sco[:], acc[t][:])
                nc.vector.copy_predicated(sco[:], maskn[:], neg[:])
                nc.sync.dma_start(scores_o[t * 128:(t + 1) * 128, :], sco[:])

                # build sort keys: trunc mantissa low10 -> embed index
                ka = kpool.tile([128, NC], F32, tag="ka")
                kb = kpool.tile([128, NC], F32, tag="kb")
                kau = ka[:].bitcast(U32)
                nc.vector.tensor_scalar(kau, sco[:].bitcast(U32), 0xFFFFFC00, scalar2=None,
                                        op0=ALU.bitwise_and)
                mpos = fin.tile([128, NC], U8, tag="mpos")
                nc.vector.tensor_scalar(mpos[:], sco[:], 0.0, scalar2=None, op0=ALU.is_ge)
                emb = fin.tile([128, NC], U32, tag="emb")
                nc.vector.tensor_copy(emb[:], tneg[:])
                nc.vector.copy_predicated(emb[:], mpos[:], tpos[:])
                nc.vector.tensor_tensor(kau, kau, emb[:], ALU.bitwise_or)

                # merge sorting network, descending, 55 stages
                cur, oth = ka, kb
                L = 1
                while L < NC:
                    nb = NC // (2 * L)
                    cv = cur[:].rearrange("p (nb two l) -> p nb two l", two=2, l=L)
                    ov = oth[:].rearrange("p (nb two l) -> p nb two l", two=2, l=L)
                    A = cv[:, :, 0, :]
                    Brev = cv[:, :, 1, ::-1]
                    nc.vector.tensor_tensor(ov[:, :, 0, :], A, Brev, ALU.max)
                    nc.vector.tensor_tensor(ov[:, :, 1, :], A, Brev, ALU.min)
                    cur, oth = oth, cur
                    j = L // 2
                    while j >= 1:
                        cv = cur[:].rearrange("p (m two j) -> p m two j", two=2, j=j)
                        ov = oth[:].rearrange("p (m two j) -> p m two j", two=2, j=j)
                        A = cv[:, :, 0, :]
                        Bv = cv[:, :, 1, :]
                        nc.vector.tensor_tensor(ov[:, :, 0, :], A, Bv, ALU.max)
                        nc.vector.tensor_tensor(ov[:, :, 1, :], A, Bv, ALU.min)
                        cur, oth = oth, cur
                        j //= 2
                    L *= 2

                # extract indices from top 512 keys
                low = fin.tile([128, TOPK], U32, tag="low")
                nc.vector.tensor_scalar(low[:], cur[:, 0:TOPK].bitcast(U32), 1023,
                                        scalar2=None, op0=ALU.bitwise_and)
                lowx = fin.tile([128, TOPK], U32, tag="lowx")
                nc.vector.tensor_scalar(lowx[:], low[:], 1023, scalar2=None, op0=ALU.bitwise_xor)
                mk = fin.tile([128, TOPK], U8, tag="mk")
                nc.vector.tensor_scalar(mk[:], cur[:, 0:TOPK], 0.0, scalar2=None, op0=ALU.is_ge)
                idx = fin.tile([128, TOPK], U32, tag="idx")
                nc.vector.tensor_copy(idx[:], low[:])
                nc.vector.copy_predicated(idx[:], mk[:], lowx[:])
                nc.sync.dma_start(topk_o[t * 128:(t + 1) * 128, :], idx[:].bitcast(I32))

    nc.compile()
    return nc


def _host_prep(prec, hidden_states, q_residual, wq, w_weights, wkv, wgate, ape, kv_norm_w):
    split = prec == "split"
    hs = np.ascontiguousarray(hidden_states.reshape(S, H), dtype=np.float32)
    qr = np.ascontiguousarray(q_residual.reshape(S, H), dtype=np.float32)
    wq = np.ascontiguousarray(wq, dtype=np.float32)
    ww = np.ascontiguousarray(w_weights, dtype=np.float32)
    wkv = np.ascontiguousarray(wkv, dtype=np.float32)
    wg = np.ascontiguousarray(wgate, dtype=np.float32)

    def shuffle_wq(x):
        # [H, NH*DH] -> [128, NSL, KT, 512]
        return np.ascontiguousarray(x.reshape(KT, 128, NSL, 512).transpose(1, 2, 0, 3))

    def shuffle_hsT(x):
        # [S, H] -> hsT[p, k, tb, c] = x[tb*512+c, k*128+p]
        return np.ascontiguousarray(x.T.reshape(KT, 128, 8, 512).transpose(1, 0, 2, 3))

    def shuffle_qrT(x):
        # per-core [512, H] chunk -> [128, KT, 512]
        return np.ascontiguousarray(x.T.reshape(KT, 128, 512).transpose(1, 0, 2))

    common = {}
    if split:
        for name, arr, shf in (("wq", wq, shuffle_wq), ("hs", hs, shuffle_hsT)):
            hi, lo = _bf16_split(arr)
            common[name + "_hi"] = shf(hi.astype(np.float32)).astype(ml_dtypes.bfloat16)
            common[name + "_lo"] = shf(lo.astype(np.float32)).astype(ml_dtypes.bfloat16)
        for name, arr, d in (("wkv", wkv, 128), ("wg", wg, 128), ("ww", ww, 64)):
            hi, lo = _bf16_split(arr)
            common[name + "_hi"] = np.ascontiguousarray(
                hi.reshape(KT, 128, d).transpose(1, 0, 2)).reshape(128, KT * d)
            common[name + "_lo"] = np.ascontiguousarray(
                lo.reshape(KT, 128, d).transpose(1, 0, 2)).reshape(128, KT * d)
    else:
        common["wq_hi"] = shuffle_wq(_rne12(wq))
        common["hs_hi"] = shuffle_hsT(_rne12(hs))
        common["wkv_hi"] = np.ascontiguousarray(
            _rne12(wkv).reshape(KT, 128, 128).transpose(1, 0, 2)).reshape(128, KT * 128)
        common["wg_hi"] = np.ascontiguousarray(
            _rne12(wg).reshape(KT, 128, 128).transpose(1, 0, 2)).reshape(128, KT * 128)
        common["ww_hi"] = np.ascontiguousarray(
            _rne12(ww).reshape(KT, 128, 64).transpose(1, 0, 2)).reshape(128, KT * 64)

    common["apet"] = np.ascontiguousarray(ape.T, dtype=np.float32)
    common["kvnw"] = np.tile(kv_norm_w.astype(np.float32)[None, :], (128, 1))
    common["ident"] = np.eye(128, dtype=np.float32)
    ck, sk = _cos_sin(np.arange(NC) * RR, THETA_K)
    common["cosk"] = np.ascontiguousarray(ck.reshape(8, 128, 32).transpose(1, 0, 2))
    common["sink"] = np.ascontiguousarray(sk.reshape(8, 128, 32).transpose(1, 0, 2))
    common["p4"] = np.tile((np.arange(NC) * RR + RR - 1).astype(np.float32)[None, :], (128, 1))
    common["tpos"] = np.tile((1023 - np.arange(NC)).astype(np.uint32)[None, :], (128, 1))
    common["tneg"] = np.tile(np.arange(NC, dtype=np.uint32)[None, :], (128, 1))

    in_maps = []
    for c in range(NCORES):
        m = dict(common)
        qchunk = qr[c * TC:(c + 1) * TC]
        if split:
            hi, lo = _bf16_split(qchunk)
            m["qr_hi"] = shuffle_qrT(hi.astype(np.float32)).astype(ml_dtypes.bfloat16)
            m["qr_lo"] = shuffle_qrT(lo.astype(np.float32)).astype(ml_dtypes.bfloat16)
        else:
            m["qr_hi"] = shuffle_qrT(_rne12(qchunk))
        pos = np.arange(c * TC, (c + 1) * TC)
        cq, sq = _cos_sin(pos, THETA_Q)
        m["cosq"] = np.ascontiguousarray(cq.reshape(NTT, 128, 32).transpose(1, 0, 2))
        m["sinq"] = np.ascontiguousarray(sq.reshape(NTT, 128, 32).transpose(1, 0, 2))
        m["tglob"] = np.ascontiguousarray(pos.reshape(NTT, 128).T.astype(np.float32))
        in_maps.append(m)
    return in_maps


def kernel(hidden_states, q_residual, wq, w_weights, wkv, wgate, ape, kv_norm_w,
           _want_results_obj=False, _trace=False):
    prec = PREC
    if prec not in _cache:
        _cache[prec] = _build_program(prec)
    nc = _cache[prec]
    in_maps = _host_prep(prec, hidden_states, q_residual, wq, w_weights, wkv, wgate,
                         ape, kv_norm_w)
    res = bass_utils.run_bass_kernel_spmd(nc, in_maps, core_ids=list(range(NCORES)),
                                          trace=_trace)
    scores = np.concatenate([r["scores_o"] for r in res.results], axis=0)[None]
    topk = np.concatenate([r["topk_o"] for r in res.results], axis=0)[None]
    if _want_results_obj:
        return (scores, topk), res
    return scores, topk
